# revision 1
# baseline (speedup 1.0000x reference)
"""DeepFM forward kernel for Trainium2, data-parallel over 8 NeuronCores.

Math refactor vs the straightforward DeepFM graph:
  sum_ij fm_interactions[b,i,j] = sum_k (sum_i m[b,i,k]) * (sum_j u[b,j,k])
so the BxNMxNU einsum collapses to an 18-dim per-row dot product of "folded"
tower outputs (16 fold products + the two additive terms via const-1 rows).
The fold is linear, so it is baked into the tower weight matrices host-side:
each tower computes [256 dense | 16 fold | 2 extras] = 274 features per row.

On-chip layout is fully transposed (features on SBUF partitions, batch on the
free dim). The tower outputs then feed the MLP matmuls directly as the moving
operand with no on-chip transposes; the inputs are transposed host-side while
sharding. All matmuls run as float32r (full-rate fp32 for free dim >= 256).

Perf structure: input DMAs ride the scalar-engine HWDGE ring while weights
ride the sync ring (parallel); dummy matmuls on a zeroed tile pre-warm the PE
HAM clock during the initial DMA window; the two towers' narrow extras
matmul groups (M=18) and the two final M=1 matmuls run concurrently in
different PE column strips via tile_position.
"""

import numpy as np

import concourse.bacc as bacc
import concourse.bass as bass  # noqa: F401
import concourse.mybir as mybir
import concourse.tile as tile
from concourse.bass_utils import run_bass_kernel_spmd

N_CORES = 8
B_FULL = 16384
R = B_FULL // N_CORES  # 2048 rows per core
F = 512                # input features per tower
KC = F // 128          # 4 contraction chunks per tower
NT = 512               # batch tile on the free dim
NTILES = R // NT       # 4
DME = 274              # tower output: 256 dense + 16 fold + 2 extras
NX = 18                # fold(16) + [add_m, 1] / [1, add_u] rows
N_WARM = 9             # PE pre-warm matmuls

F32 = mybir.dt.float32
F32R = mybir.dt.float32r

# fp32r weight-pack column offsets ([128, WCOLS] blob)
WM_OFF = 0                    # 4 * 274
WU_OFF = WM_OFF + KC * DME
W1_OFF = WU_OFF + KC * DME    # 4 * 256
W2_OFF = W1_OFF + KC * 256    # 2 * 128
W3_OFF = W2_OFF + 2 * 128     # 1
ONES_OFF = W3_OFF + 1         # 1 (first 18 partitions = 1.0)
WCOLS = ONES_OFF + 1

# fp32 bias-pack column indices ([128, BCOLS])
BM0, BM1, BMX, BU0, BU1, BUX, B1A, B1B, B2C = range(9)
BCOLS = 9


def _tower_ext(W, b, is_movie, b3=0.0):
    """[512,257],[257] -> ([512,274], [274]) with fold + extras columns.

    Extras rows after the 16 fold rows: movie tower emits [additive, const-1],
    user tower emits [const-1, additive]; the FM elementwise product of the
    two 18-row blocks then yields fold products + both additive terms, summed
    by a single ones-vector matmul. The scalar b3 rides on the movie additive
    bias so the final combine needs no separate bias.
    """
    dense_w = W[:, :256]
    fold_w = dense_w.reshape(F, 16, 16).sum(axis=1)        # [512, 16]
    add_w = W[:, 256:257]
    zero_w = np.zeros_like(add_w)
    fold_b = b[:256].reshape(16, 16).sum(axis=0)
    if is_movie:
        tail_w = [add_w, zero_w]
        tail_b = [b[256:257] + b3, np.ones(1, np.float32)]
    else:
        tail_w = [zero_w, add_w]
        tail_b = [np.ones(1, np.float32), b[256:257]]
    w_ext = np.concatenate([dense_w, fold_w, *tail_w], axis=1)
    b_ext = np.concatenate([b[:256], fold_b, *tail_b])
    return w_ext.astype(np.float32), b_ext.astype(np.float32)


def _chunk(Wext):
    """[K, M] -> [128, (K/128)*M]: K-chunk k occupies cols [k*M, (k+1)*M)."""
    kc, m = Wext.shape[0] // 128, Wext.shape[1]
    return Wext.reshape(kc, 128, m).transpose(1, 0, 2).reshape(128, kc * m)


def _col(vec):
    out = np.zeros((128, 1), np.float32)
    out[: len(vec), 0] = vec
    return out


def _pack_weights(Wm, bm, Wu, bu, W1, b1, W2, b2, W3, b3):
    b3v = float(np.asarray(b3, np.float32).reshape(1)[0])
    wm_ext, bm_ext = _tower_ext(Wm, bm, True, b3v)
    wu_ext, bu_ext = _tower_ext(Wu, bu, False)
    ones = np.zeros((128, 1), np.float32)
    ones[:NX, 0] = 1.0
    wp = np.concatenate(
        [
            _chunk(wm_ext),
            _chunk(wu_ext),
            _chunk(W1.astype(np.float32)),
            _chunk(W2.astype(np.float32)),
            W3.astype(np.float32).reshape(128, 1),
            ones,
        ],
        axis=1,
    )
    assert wp.shape == (128, WCOLS), wp.shape
    bp = np.concatenate(
        [
            _col(bm_ext[:128]), _col(bm_ext[128:256]), _col(bm_ext[256:]),
            _col(bu_ext[:128]), _col(bu_ext[128:256]), _col(bu_ext[256:]),
            _col(b1.astype(np.float32)[:128]), _col(b1.astype(np.float32)[128:]),
            _col(b2.astype(np.float32)),
        ],
        axis=1,
    )
    return np.ascontiguousarray(wp), np.ascontiguousarray(bp)


def _build_bass():
    nc = bacc.Bacc()
    xm = nc.dram_tensor("xm", [F, R], F32R, kind="ExternalInput")
    xu = nc.dram_tensor("xu", [F, R], F32R, kind="ExternalInput")
    wp = nc.dram_tensor("wp", [128, WCOLS], F32R, kind="ExternalInput")
    bp = nc.dram_tensor("bp", [128, BCOLS], F32, kind="ExternalInput")
    out = nc.dram_tensor("out", [1, R], F32, kind="ExternalOutput")

    add = mybir.AluOpType.add
    amax = mybir.AluOpType.max
    ident = mybir.ActivationFunctionType.Identity

    with tile.TileContext(nc) as tc:
        with (
            tc.tile_pool(name="wpool", bufs=1) as wpool,
            tc.tile_pool(name="xpool", bufs=3) as xpool,
            tc.tile_pool(name="dpool", bufs=2) as dpool,
            tc.tile_pool(name="opool", bufs=1) as opool,
            tc.tile_pool(name="pspool", bufs=6, space="PSUM") as pspool,
            tc.tile_pool(name="psfin", bufs=1, space="PSUM") as psfin,
            tc.tile_pool(name="pswarm", bufs=1, space="PSUM") as pswarm,
        ):
            # PE pre-warm on a zeroed tile: keeps the HAM clock-gate busy
            # through the initial DMA window so real matmuls start at 2.4 GHz.
            wgar = wpool.tile([128, NT], mybir.dt.bfloat16)
            nc.vector.memset(wgar, 0.0)
            for _ in range(N_WARM):
                pw = pswarm.tile([128, NT], F32, name="psw")
                nc.tensor.matmul(pw, wgar[:, :128], wgar, start=True, stop=True)

            # Weights ride the scalar-engine HWDGE ring in need-order
            # (wm, wu, then the MLP block) while the x tiles get the sync
            # ring to themselves — two HWDGE rings drain in parallel, and
            # neither trigger stream queues behind compute work.
            w = wpool.tile([128, WCOLS], F32R)
            nc.scalar.dma_start(out=w[:, : KC * DME], in_=wp[:, : KC * DME])
            nc.scalar.dma_start(
                out=w[:, KC * DME : 2 * KC * DME], in_=wp[:, KC * DME : 2 * KC * DME]
            )
            b = wpool.tile([128, BCOLS], F32)
            nc.scalar.dma_start(out=b, in_=bp[:, :])
            nc.scalar.dma_start(out=w[:, 2 * KC * DME :], in_=wp[:, 2 * KC * DME :])
            out_sb = opool.tile([1, R], F32)

            xmr = xm.rearrange("(c p) n -> p c n", p=128)
            xur = xu.rearrange("(c p) n -> p c n", p=128)

            for t in range(NTILES):
                n0 = t * NT
                xm_t = xpool.tile([128, KC, NT], F32R, name="xm_t")
                nc.sync.dma_start(out=xm_t, in_=xmr[:, :, n0 : n0 + NT])
                xu_t = xpool.tile([128, KC, NT], F32R, name="xu_t")
                nc.sync.dma_start(out=xu_t, in_=xur[:, :, n0 : n0 + NT])

                # --- tower dense chunks (features x batch, 4 groups) ---
                douts = {}
                for tow, (xt, woff, boff) in enumerate(
                    ((xm_t, WM_OFF, BM0), (xu_t, WU_OFF, BU0))
                ):
                    for c in range(2):
                        c0 = c * 128
                        ps = pspool.tile([128, NT], F32, name="ps_mm")
                        for k in range(KC):
                            lhsT = w[:, woff + k * DME + c0 : woff + k * DME + c0 + 128]
                            nc.tensor.matmul(
                                ps, lhsT, xt[:, k, :],
                                start=(k == 0), stop=(k == KC - 1),
                            )
                        d = dpool.tile([128, NT], F32R, name=f"d{tow}{c}")
                        nc.scalar.activation(
                            out=d, in_=ps, func=ident,
                            bias=b[:, boff + c : boff + c + 1],
                        )
                        douts[(tow, c)] = d

                # --- tower extras: two M=18 groups ---
                psxm = pspool.tile([NX, NT], F32, name="ps_mm")
                psxu = pspool.tile([NX, NT], F32, name="ps_mm")
                for k in range(KC):
                    lm = w[:, WM_OFF + k * DME + 256 : WM_OFF + k * DME + 256 + NX]
                    nc.tensor.matmul(
                        psxm, lm, xm_t[:, k, :], start=(k == 0), stop=(k == KC - 1)
                    )
                for k in range(KC):
                    lu = w[:, WU_OFF + k * DME + 256 : WU_OFF + k * DME + 256 + NX]
                    nc.tensor.matmul(
                        psxu, lu, xu_t[:, k, :], start=(k == 0), stop=(k == KC - 1)
                    )
                dmx = dpool.tile([NX, NT], F32R, name="dmx")
                nc.vector.tensor_scalar_add(out=dmx, in0=psxm, scalar1=b[:NX, BMX : BMX + 1])
                dux = dpool.tile([NX, NT], F32R, name="dux")
                nc.vector.tensor_scalar_add(out=dux, in0=psxu, scalar1=b[:NX, BUX : BUX + 1])
                prod = dpool.tile([NX, NT], F32R, name="prod")
                nc.vector.tensor_mul(out=prod, in0=dmx, in1=dux)

                # --- MLP layer 1: K = [dm0, dm1, du0, du1] ---
                dall = [douts[(0, 0)], douts[(0, 1)], douts[(1, 0)], douts[(1, 1)]]
                h1 = []
                for c in range(2):
                    ps = pspool.tile([128, NT], F32, name="ps_mm")
                    for k in range(4):
                        lhsT = w[:, W1_OFF + k * 256 + c * 128 : W1_OFF + k * 256 + (c + 1) * 128]
                        nc.tensor.matmul(ps, lhsT, dall[k], start=(k == 0), stop=(k == 3))
                    h = dpool.tile([128, NT], F32R, name=f"h1{c}")
                    if c == 0:
                        nc.scalar.activation(
                            out=h, in_=ps,
                            func=mybir.ActivationFunctionType.Relu,
                            bias=b[:, B1A + c : B1A + c + 1],
                        )
                    else:
                        nc.vector.tensor_scalar(
                            out=h, in0=ps, scalar1=b[:, B1A + c : B1A + c + 1],
                            scalar2=0.0, op0=add, op1=amax,
                        )
                    h1.append(h)

                # --- MLP layer 2 ---
                ps = pspool.tile([128, NT], F32, name="ps_mm")
                for k in range(2):
                    lhsT = w[:, W2_OFF + k * 128 : W2_OFF + (k + 1) * 128]
                    nc.tensor.matmul(ps, lhsT, h1[k], start=(k == 0), stop=(k == 1))
                h2 = dpool.tile([128, NT], F32R, name="h2")
                nc.scalar.activation(
                    out=h2, in_=ps,
                    func=mybir.ActivationFunctionType.Relu,
                    bias=b[:, B2C : B2C + 1],
                )

                # --- final: logit = W3.T @ h2 + ones18.T @ prod (b3 rides in
                # the additive bias) ---
                psf = psfin.tile([1, NT], F32, name="ps_fin")
                nc.tensor.matmul(
                    psf, w[:, W3_OFF : W3_OFF + 1], h2, start=True, stop=False
                )
                nc.tensor.matmul(
                    psf, w[:NX, ONES_OFF : ONES_OFF + 1], prod, start=False, stop=True
                )
                nc.vector.tensor_copy(out_sb[:, n0 : n0 + NT], psf)
                nc.scalar.dma_start(
                    out=out[:, n0 : n0 + NT], in_=out_sb[:, n0 : n0 + NT]
                )
    nc.finalize()
    return nc


_NC_CACHE = []


def kernel(movie_vectors, user_vectors, Wm, bm, Wu, bu, W1, b1, W2, b2, W3, b3):
    movie_vectors = np.asarray(movie_vectors, np.float32)
    user_vectors = np.asarray(user_vectors, np.float32)
    wp, bp = _pack_weights(
        np.asarray(Wm, np.float32), np.asarray(bm, np.float32),
        np.asarray(Wu, np.float32), np.asarray(bu, np.float32),
        np.asarray(W1, np.float32), np.asarray(b1, np.float32),
        np.asarray(W2, np.float32), np.asarray(b2, np.float32),
        np.asarray(W3, np.float32), np.asarray(b3, np.float32),
    )
    xmT = np.ascontiguousarray(movie_vectors.T)  # [512, 16384]
    xuT = np.ascontiguousarray(user_vectors.T)

    if not _NC_CACHE:
        _NC_CACHE.append(_build_bass())
    nc = _NC_CACHE[0]

    in_maps = []
    for c in range(N_CORES):
        sl = slice(c * R, (c + 1) * R)
        in_maps.append(
            {
                "xm": np.ascontiguousarray(xmT[:, sl]),
                "xu": np.ascontiguousarray(xuT[:, sl]),
                "wp": wp,
                "bp": bp,
            }
        )
    res = run_bass_kernel_spmd(nc, in_maps, core_ids=list(range(N_CORES)))
    kernel.last_result = res
    return np.concatenate([r["out"].reshape(R, 1) for r in res.results], axis=0)



# revision 4
# speedup vs baseline: 1.2972x; 1.2972x over previous
"""DeepFM forward kernel for Trainium2, data-parallel over 8 NeuronCores.

Math refactor vs the straightforward DeepFM graph:
  1. The 256-wide tower dense outputs are only consumed by (a) the FM
     interaction sum and (b) MLP layer 1. (a) collapses to 16 "fold" sums
     per tower (sum_ij m_i.u_j = sum_k (sum_i m_ik)(sum_j u_jk)) and (b) is
     linear, so W1 is folded into the tower weights host-side:
         z1 = xm @ (Wm_d @ W1[:256]) + xu @ (Wu_d @ W1[256:]) + b1'
     The dense tower outputs are never materialized on chip.
  2. The FM sum uses the polarization identity
         sum_k fold_m.fold_u + add = sum_k (p_k^2 - q_k^2)/4 + a
     with p = fold_m + fold_u, q = fold_m - fold_u (both linear in x), and
     the additive term a rides rows 32/33 as ((a+1)/2)^2 - ((a-1)/2)^2 = a.
     So the whole FM side is ONE 34-row matmul accumulation chain plus ONE
     scalar-engine Square activation, then folds into the final matmul.

Everything runs in bf16 (inputs quantized host-side; fp32 PSUM accumulate),
which halves DMA traffic and enables FWL fast weight loads on the PE.
Inputs are repacked host-side so each batch tile is one fully-contiguous
4KB-per-partition DMA.
"""

import numpy as np
import ml_dtypes

import concourse.bacc as bacc
import concourse.bass as bass  # noqa: F401
import concourse.mybir as mybir
import concourse.tile as tile
from concourse.bass_utils import run_bass_kernel_spmd

N_CORES = 8
B_FULL = 16384
R = B_FULL // N_CORES  # 2048 rows per core
F = 512                # input features per tower
KC = F // 128          # 4 contraction chunks per tower
NT = 512               # batch tile on the free dim
NTILES = R // NT       # 4
NX = 34                # extras rows: p(16) + q(16) + a-rows(2)
N_WARM = 9             # PE pre-warm matmuls

F32 = mybir.dt.float32
BF16 = mybir.dt.bfloat16

# bf16 weight-pack column offsets ([128, WCOLS] blob)
Z1_OFF = 0                      # 16 x 128: (g, j) at (g*8+j)*128; j=0-3 Am, 4-7 Au
Z1_COLS = 16 * 128
X_OFF = Z1_COLS                 # 8 x 34: j=0-3 Xm chunks, 4-7 Xu chunks
W2_OFF = X_OFF + 8 * NX         # 2 x 128
W3_OFF = W2_OFF + 2 * 128       # 1
WQ_OFF = W3_OFF + 1             # 1 (rows 0-33 = [1/4]*16 + [-1/4]*16 + [1, -1])
WCOLS = WQ_OFF + 1

# fp32 bias-pack column indices ([128, BCOLS])
B1A, B1B, BX, B2C = range(4)
BCOLS = 4


def _chunk(Wext):
    """[K, M] -> [128, (K/128)*M]: K-chunk k occupies cols [k*M, (k+1)*M)."""
    kc, m = Wext.shape[0] // 128, Wext.shape[1]
    return Wext.reshape(kc, 128, m).transpose(1, 0, 2).reshape(128, kc * m)


def _col(vec):
    out = np.zeros((128, 1), np.float32)
    out[: len(vec), 0] = vec
    return out


def _pack_weights(Wm, bm, Wu, bu, W1, b1, W2, b2, W3, b3):
    f64 = np.float64
    Wm, bm, Wu, bu = Wm.astype(f64), bm.astype(f64), Wu.astype(f64), bu.astype(f64)
    W1, b1, W2, b2 = W1.astype(f64), b1.astype(f64), W2.astype(f64), b2.astype(f64)
    b3v = float(np.asarray(b3, f64).reshape(-1)[0])

    # fused z1 = xm @ Am + xu @ Au + b1p
    Am = Wm[:, :256] @ W1[:256, :]
    Au = Wu[:, :256] @ W1[256:, :]
    b1p = b1 + bm[:256] @ W1[:256, :] + bu[:256] @ W1[256:, :]

    # FM extras: p/q fold rows + additive rows
    FWm = Wm[:, :256].reshape(F, 16, 16).sum(axis=1)  # sum over i -> [512, 16]
    FWu = Wu[:, :256].reshape(F, 16, 16).sum(axis=1)
    fbm = bm[:256].reshape(16, 16).sum(axis=0)
    fbu = bu[:256].reshape(16, 16).sum(axis=0)
    awm, awu = Wm[:, 256], Wu[:, 256]
    A = bm[256] + bu[256] + b3v
    Xm = np.concatenate([FWm, FWm, awm[:, None] / 2, awm[:, None] / 2], axis=1)
    Xu = np.concatenate([FWu, -FWu, awu[:, None] / 2, awu[:, None] / 2], axis=1)
    xbias = np.concatenate([fbm + fbu, fbm - fbu, [(A + 1) / 2], [(A - 1) / 2]])
    wq = np.concatenate([np.full(16, 0.25), np.full(16, -0.25), [1.0, -1.0]])

    # z1 block in accumulation order: group g, then xm chunks 0-3, xu chunks 0-3
    amc, auc = _chunk(Am), _chunk(Au)  # [128, 4*256]
    z1_cols = []
    for g in range(2):
        for k in range(KC):
            z1_cols.append(amc[:, k * 256 + g * 128 : k * 256 + (g + 1) * 128])
        for k in range(KC):
            z1_cols.append(auc[:, k * 256 + g * 128 : k * 256 + (g + 1) * 128])
    wq_col = np.zeros((128, 1), f64)
    wq_col[:NX, 0] = wq
    wp = np.concatenate(
        z1_cols
        + [_chunk(Xm), _chunk(Xu), _chunk(W2), np.asarray(W3, f64).reshape(128, 1), wq_col],
        axis=1,
    )
    assert wp.shape == (128, WCOLS), wp.shape
    bp = np.concatenate(
        [_col(b1p[:128]), _col(b1p[128:]), _col(xbias), _col(b2)], axis=1
    )
    return (
        np.ascontiguousarray(wp.astype(ml_dtypes.bfloat16)),
        np.ascontiguousarray(bp.astype(np.float32)),
    )


def _build_bass():
    nc = bacc.Bacc()
    xm = nc.dram_tensor("xm", [128, NTILES * KC * NT], BF16, kind="ExternalInput")
    xu = nc.dram_tensor("xu", [128, NTILES * KC * NT], BF16, kind="ExternalInput")
    wp = nc.dram_tensor("wp", [128, WCOLS], BF16, kind="ExternalInput")
    bp = nc.dram_tensor("bp", [128, BCOLS], F32, kind="ExternalInput")
    out = nc.dram_tensor("out", [1, R], F32, kind="ExternalOutput")

    relu = mybir.ActivationFunctionType.Relu
    square = mybir.ActivationFunctionType.Square

    with tile.TileContext(nc) as tc:
        with (
            tc.tile_pool(name="wpool", bufs=1) as wpool,
            tc.tile_pool(name="xpool", bufs=1) as xpool,
            tc.tile_pool(name="dpool", bufs=1) as dpool,
            tc.tile_pool(name="opool", bufs=1) as opool,
            tc.tile_pool(name="psz", bufs=4, space="PSUM") as psz,
            tc.tile_pool(name="psx", bufs=2, space="PSUM") as psx,
            tc.tile_pool(name="psm", bufs=1, space="PSUM") as psm,
            tc.tile_pool(name="psf", bufs=1, space="PSUM") as psf,
        ):
            # PE pre-warm on a zeroed tile: covers the HAM activity window
            # during the initial DMA so real matmuls start at 2.4 GHz.
            wgar = wpool.tile([128, NT], BF16)
            nc.vector.memset(wgar, 0.0)
            for _ in range(N_WARM):
                pw = psm.tile([128, NT], F32, name="ps_m")
                nc.tensor.matmul(pw, wgar[:, :128], wgar, start=True, stop=True)

            # Weights on the scalar HWDGE ring (inputs get the sync ring):
            # z1 block first (needed by the first matmuls), then the rest.
            w = wpool.tile([128, WCOLS], BF16)
            nc.scalar.dma_start(out=w[:, :Z1_COLS], in_=wp[:, :Z1_COLS])
            b = wpool.tile([128, BCOLS], F32)
            nc.scalar.dma_start(out=b, in_=bp[:, :])
            nc.scalar.dma_start(out=w[:, Z1_COLS:], in_=wp[:, Z1_COLS:])
            out_sb = opool.tile([1, R], F32)

            xmr = xm.rearrange("p (t c n) -> p t c n", t=NTILES, c=KC, n=NT)
            xur = xu.rearrange("p (t c n) -> p t c n", t=NTILES, c=KC, n=NT)

            # All input tiles prefetch up front; each is one fully-contiguous
            # 4KB-per-partition transfer.
            xts = []
            for t in range(NTILES):
                xm_t = xpool.tile([128, KC, NT], BF16, name=f"xm{t}")
                nc.sync.dma_start(out=xm_t, in_=xmr[:, t])
                xu_t = xpool.tile([128, KC, NT], BF16, name=f"xu{t}")
                nc.sync.dma_start(out=xu_t, in_=xur[:, t])
                xts.append((xm_t, xu_t))

            h1s, sqs, h2s = {}, {}, {}

            def emit_z1(t):
                xm_t, xu_t = xts[t]
                for g in range(2):
                    ps = psz.tile([128, NT], F32, name="ps_z1")
                    for j in range(8):
                        xt, k = (xm_t, j) if j < 4 else (xu_t, j - 4)
                        lhsT = w[:, (g * 8 + j) * 128 : (g * 8 + j + 1) * 128]
                        nc.tensor.matmul(
                            ps, lhsT, xt[:, k, :], start=(j == 0), stop=(j == 7)
                        )
                    h = dpool.tile([128, NT], BF16, name=f"h1_{t}_{g}")
                    nc.scalar.activation(
                        out=h, in_=ps, func=relu, bias=b[:, g : g + 1]
                    )
                    h1s[(t, g)] = h

            def emit_extras(t):
                xm_t, xu_t = xts[t]
                ps = psx.tile([NX, NT], F32, name="ps_x")
                for j in range(8):
                    xt, k = (xm_t, j) if j < 4 else (xu_t, j - 4)
                    lhsT = w[:, X_OFF + j * NX : X_OFF + (j + 1) * NX]
                    nc.tensor.matmul(
                        ps, lhsT, xt[:, k, :], start=(j == 0), stop=(j == 7)
                    )
                sq = dpool.tile([NX, NT], BF16, name=f"sq_{t}")
                nc.scalar.activation(
                    out=sq, in_=ps, func=square, bias=b[:NX, BX : BX + 1]
                )
                sqs[t] = sq

            def emit_mlp2(t):
                ps = psm.tile([128, NT], F32, name="ps_m")
                for c in range(2):
                    lhsT = w[:, W2_OFF + c * 128 : W2_OFF + (c + 1) * 128]
                    nc.tensor.matmul(
                        ps, lhsT, h1s[(t, c)], start=(c == 0), stop=(c == 1)
                    )
                h2 = dpool.tile([128, NT], BF16, name=f"h2_{t}")
                nc.scalar.activation(
                    out=h2, in_=ps, func=relu, bias=b[:, B2C : B2C + 1]
                )
                h2s[t] = h2

            def emit_final(t):
                ps = psf.tile([1, NT], F32, name="ps_f")
                nc.tensor.matmul(
                    ps, w[:, W3_OFF : W3_OFF + 1], h2s[t], start=True, stop=False
                )
                nc.tensor.matmul(
                    ps, w[:NX, WQ_OFF : WQ_OFF + 1], sqs[t], start=False, stop=True
                )
                n0 = t * NT
                nc.vector.tensor_copy(out_sb[:, n0 : n0 + NT], ps)
                nc.scalar.dma_start(
                    out=out[:, n0 : n0 + NT], in_=out_sb[:, n0 : n0 + NT]
                )

            for t in range(NTILES):
                emit_z1(t)
                if t > 0:
                    emit_final(t - 1)
                emit_extras(t)
                emit_mlp2(t)
            emit_final(NTILES - 1)
    nc.finalize()
    return nc


def _pack_x(xT_core):
    """[512, 2048] fp32 -> [128, NTILES*KC*NT] bf16, tile-contiguous."""
    y = (
        xT_core.reshape(KC, 128, NTILES, NT)  # [c, p, t, n]
        .transpose(1, 2, 0, 3)                # [p, t, c, n]
        .reshape(128, NTILES * KC * NT)
    )
    return np.ascontiguousarray(y.astype(ml_dtypes.bfloat16))


_NC_CACHE = []


def kernel(movie_vectors, user_vectors, Wm, bm, Wu, bu, W1, b1, W2, b2, W3, b3):
    movie_vectors = np.asarray(movie_vectors, np.float32)
    user_vectors = np.asarray(user_vectors, np.float32)
    wp, bp = _pack_weights(
        np.asarray(Wm, np.float32), np.asarray(bm, np.float32),
        np.asarray(Wu, np.float32), np.asarray(bu, np.float32),
        np.asarray(W1, np.float32), np.asarray(b1, np.float32),
        np.asarray(W2, np.float32), np.asarray(b2, np.float32),
        np.asarray(W3, np.float32), np.asarray(b3, np.float32),
    )
    xmT = movie_vectors.T  # [512, 16384]
    xuT = user_vectors.T

    if not _NC_CACHE:
        _NC_CACHE.append(_build_bass())
    nc = _NC_CACHE[0]

    in_maps = []
    for c in range(N_CORES):
        sl = slice(c * R, (c + 1) * R)
        in_maps.append(
            {
                "xm": _pack_x(xmT[:, sl]),
                "xu": _pack_x(xuT[:, sl]),
                "wp": wp,
                "bp": bp,
            }
        )
    res = run_bass_kernel_spmd(nc, in_maps, core_ids=list(range(N_CORES)))
    kernel.last_result = res
    return np.concatenate([r["out"].reshape(R, 1) for r in res.results], axis=0)
